# revision 3
# baseline (speedup 1.0000x reference)
"""Toeplitz bias kernel for trn2 (8 NeuronCores).

bias[h, j, i] = exp(w_[h] - offset[h])[2*L-2 + j - i]   with L = 2048.

Let q = reverse(exp(w_ - offset)) (length S = 2*L-1 = 4095); then
bias[h, j, i] = q[L-1 - j + i].

Device strategy (no staircase, no chained copies):
  1. One broadcast-load DMA per head replicates the packed row
     [w_rev (4095) | -offset] into all 128 SBUF partitions. The host
     ships NREP=16 identical copies of the row so the load's stride-0
     source fans across distinct HBM regions (a single-region stride-0
     re-read is DRAM-channel-bound and ~2.5x slower).
  2. One activation per head computes qe = exp(w_rev + (-offset)) over
     the whole [128, 4095] tile (bias operand = the -offset column).
  3. Stores read the replicated tile through a *diagonal* access
     pattern: dim0 stride = pitch-1 makes partition t start one element
     earlier, i.e. src[t, i] = qe[t, c0 - t + i] with c0 = L-1-128b --
     exactly the Toeplitz row shift, straight out of SBUF addressing.
     NB consecutive 128-row blocks are fused into one DMA by iterating
     blocks descending: the SBUF column base ascends (+128, legal) while
     the DRAM block offset descends (-128*L, negative strides are fine
     on the DRAM side). Fewer, larger stores keep ~16 MB in flight per
     HWDGE queue and the fabric saturated.

Heads are sharded 2 per core across 8 cores; the host concatenates the
per-core [2, L, L] outputs.
"""

import numpy as np

H = 16
L = 2048
S = 2 * L - 1  # 4095
N_CORES = 8
HPC = H // N_CORES  # heads per core
NBLK = L // 128  # 16 row blocks per head
NREP = 16  # host-side row replication for the broadcast load

_cached_nc = None


def _build_nc(variant="mb4"):
    import bass_rust
    import concourse.bacc as bacc
    import concourse.mybir as mybir
    import concourse.tile as tile

    nb = int(variant[2])  # blocks fused per store DMA
    q3 = variant.endswith("q3")
    ngrp = NBLK // nb

    nc = bacc.Bacc("TRN2", target_bir_lowering=False)
    f32 = mybir.dt.float32
    # win[h, r] = [reversed w_ row | -offset], replicated NREP times
    win = nc.dram_tensor("win", [HPC, NREP, S + 1], f32, kind="ExternalInput")
    out = nc.dram_tensor("out", [HPC, L, L], f32, kind="ExternalOutput")

    with tile.TileContext(nc) as tc:
        with tc.tile_pool(name="p", bufs=1) as pool:
            wts, qes = [], []
            for h in range(HPC):
                wt = pool.tile([128, S + 1], f32, tag=f"wt{h}")
                qe = pool.tile([128, S], f32, tag=f"qe{h}")
                wts.append(wt)
                qes.append(qe)
                # replicated broadcast load: partition p <- win[h, p % NREP]
                src = win[h : h + 1, 0:1, :].partition_broadcast(128 // NREP)
                src.ap = bass_rust.VecI64Pair(
                    [[0, 128 // NREP], [S + 1, NREP], [1, S + 1]]
                )
                (nc.sync, nc.gpsimd)[h % 2].dma_start(wt[:, :], src)
            for h in range(HPC):
                nc.scalar.activation(
                    qes[h][:, :],
                    wts[h][:, 0:S],
                    mybir.ActivationFunctionType.Exp,
                    bias=wts[h][:, S : S + 1],
                )
            k = 0
            for h in range(HPC):
                for g in range(ngrp):
                    b_hi = g * nb + nb - 1
                    c0 = L - 1 - 128 * b_hi
                    sap = qes[h][:, 0:L]
                    dst = out[h, 0:128, :]
                    if nb == 1:
                        sap.ap = bass_rust.VecI64Pair([[S - 1, 128], [1, L]])
                        sap.offset = c0
                        dst = out[h, 128 * b_hi : 128 * (b_hi + 1), :]
                    else:
                        sap.ap = bass_rust.VecI64Pair(
                            [[S - 1, 128], [128, nb], [1, L]]
                        )
                        sap.offset = c0
                        dst.ap = bass_rust.VecI64Pair(
                            [[L, 128], [-128 * L, nb], [1, L]]
                        )
                        dst.offset = (h * L + 128 * b_hi) * L
                    engs = (
                        (nc.sync, nc.scalar, nc.gpsimd)
                        if q3
                        else (nc.sync, nc.scalar)
                    )
                    engs[k % len(engs)].dma_start(dst, sap)
                    k += 1
    nc.compile()
    return nc


def _get_nc():
    global _cached_nc
    if _cached_nc is None:
        _cached_nc = _build_nc()
    return _cached_nc


def _make_in_maps(w_, offset):
    w_ = np.asarray(w_, dtype=np.float32)
    offset = np.asarray(offset, dtype=np.float32)
    win = np.empty((H, NREP, S + 1), dtype=np.float32)
    win[:, 0, 0:S] = w_[:, ::-1]
    win[:, 0, S] = -offset
    win[:, 1:, :] = win[:, 0:1, :]
    in_maps = []
    for c in range(N_CORES):
        sl = slice(c * HPC, (c + 1) * HPC)
        in_maps.append({"win": np.ascontiguousarray(win[sl])})
    return in_maps


def run(w_, offset, trace=False, variant="mb4", **trace_kw):
    import concourse.bass_utils as bu
    from concourse.bass_utils import run_bass_kernel_spmd

    if trace:
        # no fish bucket in this container; keep artifacts local
        bu.upload_artifacts = lambda tmpdir: "local://" + str(tmpdir)

    if variant == "mb4":
        nc = _get_nc()
    else:
        nc = _build_nc(variant)
    in_maps = _make_in_maps(w_, offset)
    res = run_bass_kernel_spmd(
        nc, in_maps, list(range(N_CORES)), trace=trace, **trace_kw
    )
    parts = [np.asarray(r["out"]) for r in res.results]
    full = np.concatenate(parts, axis=0)  # [H, L, L]
    return full, res


def kernel(w_, offset, seq_len=None, **_ignored):
    full, _ = run(w_, offset, trace=False)
    return full


# revision 4
# speedup vs baseline: 1.2400x; 1.2400x over previous
"""Toeplitz bias kernel for trn2 (8 NeuronCores).

bias[h, j, i] = exp(w_[h] - offset[h])[2*L-2 + j - i]   with L = 2048.

Let q = reverse(exp(w_ - offset)) (length S = 2*L-1 = 4095); then
bias[h, j, i] = q[L-1 - j + i].

Device strategy (no staircase, no chained copies):
  1. One plain 2 MB load per head brings [w_rev | -offset] pre-replicated
     across 128 rows (host-side) into a [128, 4096] SBUF tile.
  2. One activation per head computes qe = exp(w_rev + (-offset)) over the
     whole [128, S] tile (bias operand = the -offset column).
  3. Stores read the replicated tile through a *diagonal* access pattern:
     giving dim0 a stride of (pitch - d) makes partition t start d
     elements earlier -- the Toeplitz row shift comes straight out of
     SBUF addressing, no data movement. With d=4 (variant r4x*),
     partition t covers output row j = 512*sb + 4t + r, so per-partition
     row starts stay 16-byte aligned and uniform; four r-phase DMAs per
     512-row super-block.

Heads are sharded 2 per core across 8 cores; the host concatenates the
per-core [2, L, L] outputs.
"""

import numpy as np

H = 16
L = 2048
S = 2 * L - 1  # 4095
N_CORES = 8
HPC = H // N_CORES  # heads per core
P = S + 1  # tile pitch (4096)

_cached_nc = None
DEFAULT_VARIANT = "r4x16"


def _build_nc(variant=DEFAULT_VARIANT):
    import bass_rust
    import concourse.bacc as bacc
    import concourse.mybir as mybir
    import concourse.tile as tile

    nc = bacc.Bacc("TRN2", target_bir_lowering=False)
    f32 = mybir.dt.float32
    win = nc.dram_tensor("win", [HPC, 128, P], f32, kind="ExternalInput")
    out = nc.dram_tensor("out", [HPC, L, L], f32, kind="ExternalOutput")

    with tile.TileContext(nc) as tc:
        with tc.tile_pool(name="p", bufs=1) as pool:
            wts, qes = [], []
            for h in range(HPC):
                wt = pool.tile([128, P], f32, tag=f"wt{h}")
                qe = pool.tile([128, P], f32, tag=f"qe{h}")
                wts.append(wt)
                qes.append(qe)
                (nc.sync, nc.scalar)[h % 2].dma_start(wt[:, :], win[h])
            for h in range(HPC):
                nc.scalar.activation(
                    qes[h][:, 0:S],
                    wts[h][:, 0:S],
                    mybir.ActivationFunctionType.Exp,
                    bias=wts[h][:, S : S + 1],
                )

            k = 0

            def store(dst, sap):
                nonlocal k
                (nc.sync, nc.scalar)[k % 2].dma_start(dst, sap)
                k += 1

            if variant == "r1x16":
                # 16 stores/head of [128, L]; per-partition shift 1 elem
                for h in range(HPC):
                    for b in range(16):
                        c0 = L - 1 - 128 * b
                        sap = qes[h][:, 0:L]
                        sap.ap = bass_rust.VecI64Pair([[P - 1, 128], [1, L]])
                        sap.offset = c0
                        store(out[h, 128 * b : 128 * (b + 1), :], sap)
            elif variant == "r4x16":
                # 16 stores/head; partition t -> row 512*sb + 4t + r
                for h in range(HPC):
                    for sb in range(4):
                        for r in range(4):
                            c0 = L - 1 - 512 * sb - r
                            sap = qes[h][:, 0:L]
                            sap.ap = bass_rust.VecI64Pair([[P - 4, 128], [1, L]])
                            sap.offset = c0
                            dst = out[h, 0:128, :]
                            dst.ap = bass_rust.VecI64Pair([[4 * L, 128], [1, L]])
                            dst.offset = (h * L + 512 * sb + r) * L
                            store(dst, sap)
            elif variant == "r4x4":
                # 4 stores/head of 4 MB; sb fused via descending DRAM stride
                for h in range(HPC):
                    for r in range(4):
                        c0 = L - 1 - 512 * 3 - r
                        sap = qes[h][:, 0:L]
                        sap.ap = bass_rust.VecI64Pair(
                            [[P - 4, 128], [512, 4], [1, L]]
                        )
                        sap.offset = c0
                        dst = out[h, 0:128, :]
                        dst.ap = bass_rust.VecI64Pair(
                            [[4 * L, 128], [-512 * L, 4], [1, L]]
                        )
                        dst.offset = (h * L + 512 * 3 + r) * L
                        store(dst, sap)
            else:
                raise ValueError(variant)
    nc.compile()
    return nc


def _get_nc():
    global _cached_nc
    if _cached_nc is None:
        _cached_nc = _build_nc()
    return _cached_nc


def _make_in_maps(w_, offset):
    w_ = np.asarray(w_, dtype=np.float32)
    offset = np.asarray(offset, dtype=np.float32)
    row = np.empty((H, P), dtype=np.float32)
    row[:, 0:S] = w_[:, ::-1]
    row[:, S] = -offset
    win = np.broadcast_to(row[:, None, :], (H, 128, P))
    in_maps = []
    for c in range(N_CORES):
        sl = slice(c * HPC, (c + 1) * HPC)
        in_maps.append({"win": np.ascontiguousarray(win[sl])})
    return in_maps


def run(w_, offset, trace=False, variant=DEFAULT_VARIANT, **trace_kw):
    import concourse.bass_utils as bu
    from concourse.bass_utils import run_bass_kernel_spmd

    if trace:
        # no fish bucket in this container; keep artifacts local
        bu.upload_artifacts = lambda tmpdir: "local://" + str(tmpdir)

    if variant == DEFAULT_VARIANT:
        nc = _get_nc()
    else:
        nc = _build_nc(variant)
    in_maps = _make_in_maps(w_, offset)
    res = run_bass_kernel_spmd(
        nc, in_maps, list(range(N_CORES)), trace=trace, **trace_kw
    )
    parts = [np.asarray(r["out"]) for r in res.results]
    full = np.concatenate(parts, axis=0)  # [H, L, L]
    return full, res


def kernel(w_, offset, seq_len=None, **_ignored):
    full, _ = run(w_, offset, trace=False)
    return full


# revision 5
# speedup vs baseline: 1.2714x; 1.0254x over previous
"""Toeplitz bias kernel for trn2 (8 NeuronCores).

bias[h, j, i] = exp(w_[h] - offset[h])[2*L-2 + j - i]   with L = 2048.

Let q = reverse(exp(w_ - offset)) (length S = 2*L-1 = 4095); then
bias[h, j, i] = q[L-1 - j + i].

Device pipeline per head (no staircase, no chained small copies):
  1. load the packed 16 KB row [w_rev | -offset] into SBUF partition 0;
  2. exp on ACT over [1, S] (activation time is column-bound, so one
     partition costs the same as 128) with bias = -offset;
  3. gpsimd partition_broadcast replicates the exp'd row into a
     [128, S] tile -- engine-side, no DMA/HBM traffic;
  4. stores read that tile through a *diagonal* access pattern: giving
     dim0 a stride of (pitch - 4) makes partition t start 4 elements
     (16 B, line-aligned) earlier, so partition t supplies output row
     j = 512*sb + 4t + r and a [128, L] block store is one DMA:
        src[t, i] = qe[t, (L-1-512sb-r) - 4t + i]
     Four r-phases x four super-blocks = 16 store DMAs per head,
     alternated across the two HWDGE queues, ~16 MB in flight each.

The store phase is HBM-bound (~390 GB/s/core with all 8 cores writing);
everything else is off the critical path except ~10 us of load+exp+bcast.

Heads are sharded 2 per core across 8 cores; the host concatenates the
per-core [2, L, L] outputs.
"""

import numpy as np

H = 16
L = 2048
S = 2 * L - 1  # 4095
N_CORES = 8
HPC = H // N_CORES  # heads per core
P = S + 1  # tile pitch (4096)

_cached_nc = None
DEFAULT_VARIANT = "pb_r4x16"


def _build_nc(variant=DEFAULT_VARIANT):
    import bass_rust
    import concourse.bacc as bacc
    import concourse.mybir as mybir
    import concourse.tile as tile

    host_exp = variant.startswith("hx")
    q3 = variant.endswith("q3")
    fused = "r4x4" in variant

    nc = bacc.Bacc("TRN2", target_bir_lowering=False)
    f32 = mybir.dt.float32
    win = nc.dram_tensor("win", [HPC, P], f32, kind="ExternalInput")
    out = nc.dram_tensor("out", [HPC, L, L], f32, kind="ExternalOutput")

    with tile.TileContext(nc) as tc:
        with tc.tile_pool(name="p", bufs=1) as pool:
            qes = []
            for h in range(HPC):
                wt = pool.tile([1, P], f32, tag=f"wt{h}")
                qe = pool.tile([128, P], f32, tag=f"qe{h}")
                qes.append(qe)
                (nc.sync, nc.scalar)[h % 2].dma_start(
                    wt[:, :], win[h : h + 1, :]
                )
                if host_exp:
                    nc.gpsimd.partition_broadcast(qe[:, 0:S], wt[0:1, 0:S])
                else:
                    q1 = pool.tile([1, S], f32, tag=f"q1{h}")
                    nc.scalar.activation(
                        q1[:, :],
                        wt[0:1, 0:S],
                        mybir.ActivationFunctionType.Exp,
                        bias=wt[0:1, S : S + 1],
                    )
                    nc.gpsimd.partition_broadcast(qe[:, 0:S], q1[0:1, :])

            k = 0

            def store(dst, sap):
                nonlocal k
                engs = (
                    (nc.sync, nc.scalar, nc.gpsimd)
                    if q3
                    else (nc.sync, nc.scalar)
                )
                engs[k % len(engs)].dma_start(dst, sap)
                k += 1

            for h in range(HPC):
                if fused:
                    # 4 stores/head of 4 MB: sb fused, descending DRAM stride
                    for r in range(4):
                        c0 = L - 1 - 512 * 3 - r
                        sap = qes[h][:, 0:L]
                        sap.ap = bass_rust.VecI64Pair(
                            [[P - 4, 128], [512, 4], [1, L]]
                        )
                        sap.offset = c0
                        dst = out[h, 0:128, :]
                        dst.ap = bass_rust.VecI64Pair(
                            [[4 * L, 128], [-512 * L, 4], [1, L]]
                        )
                        dst.offset = (h * L + 512 * 3 + r) * L
                        store(dst, sap)
                else:
                    # 16 stores/head; partition t -> row 512*sb + 4t + r
                    for sb in range(4):
                        for r in range(4):
                            c0 = L - 1 - 512 * sb - r
                            sap = qes[h][:, 0:L]
                            sap.ap = bass_rust.VecI64Pair([[P - 4, 128], [1, L]])
                            sap.offset = c0
                            dst = out[h, 0:128, :]
                            dst.ap = bass_rust.VecI64Pair([[4 * L, 128], [1, L]])
                            dst.offset = (h * L + 512 * sb + r) * L
                            store(dst, sap)
    nc.compile()
    return nc


def _get_nc():
    global _cached_nc
    if _cached_nc is None:
        _cached_nc = _build_nc()
    return _cached_nc


def _make_in_maps(w_, offset, host_exp=False):
    w_ = np.asarray(w_, dtype=np.float32)
    offset = np.asarray(offset, dtype=np.float32)
    win = np.zeros((H, P), dtype=np.float32)
    if host_exp:
        win[:, 0:S] = np.exp(w_[:, ::-1] - offset[:, None])
    else:
        win[:, 0:S] = w_[:, ::-1]
        win[:, S] = -offset
    in_maps = []
    for c in range(N_CORES):
        sl = slice(c * HPC, (c + 1) * HPC)
        in_maps.append({"win": np.ascontiguousarray(win[sl])})
    return in_maps


def run(w_, offset, trace=False, variant=DEFAULT_VARIANT, **trace_kw):
    import concourse.bass_utils as bu
    from concourse.bass_utils import run_bass_kernel_spmd

    if trace:
        # no fish bucket in this container; keep artifacts local
        bu.upload_artifacts = lambda tmpdir: "local://" + str(tmpdir)

    if variant == DEFAULT_VARIANT:
        nc = _get_nc()
    else:
        nc = _build_nc(variant)
    in_maps = _make_in_maps(w_, offset, host_exp=variant.startswith("hx"))
    res = run_bass_kernel_spmd(
        nc, in_maps, list(range(N_CORES)), trace=trace, **trace_kw
    )
    parts = [np.asarray(r["out"]) for r in res.results]
    full = np.concatenate(parts, axis=0)  # [H, L, L]
    return full, res


def kernel(w_, offset, seq_len=None, **_ignored):
    full, _ = run(w_, offset, trace=False)
    return full


# revision 7
# speedup vs baseline: 1.3945x; 1.0968x over previous
"""Toeplitz bias kernel for trn2 (8 NeuronCores).

bias[h, j, i] = exp(w_[h] - offset[h])[2*L-2 + j - i]   with L = 2048.

Let q = reverse(exp(w_ - offset)) (length S = 2*L-1 = 4095); then
bias[h, j, i] = q[L-1 - j + i].

Device pipeline per head (no staircase, no chained small copies):
  1. load the packed 16 KB row [w_rev | -offset] into SBUF partition 0;
  2. exp on ACT over [1, S] (activation time is column-bound, so one
     partition costs the same as 128) with bias = -offset;
  3. gpsimd partition_broadcast replicates the exp'd row into a
     [128, S] tile -- engine-side, no DMA/HBM traffic;
  4. stores read that tile through a *diagonal* access pattern: giving
     dim0 a stride of (pitch - 4) makes partition t start 4 elements
     (16 B, line-aligned) earlier, so partition t supplies output row
     j = 512*sb + 4t + r and a [128, L] block store is one DMA:
        src[t, i] = qe[t, (L-1-512sb-r) - 4t + i]
     Four r-phases x four super-blocks = 16 store DMAs per head,
     alternated across the two HWDGE queues, ~16 MB in flight each.

The store phase is HBM-bound (~390 GB/s/core with all 8 cores writing);
everything else is off the critical path except ~10 us of load+exp+bcast.

Heads are sharded 2 per core across 8 cores; the host concatenates the
per-core [2, L, L] outputs.
"""

import numpy as np

H = 16
L = 2048
S = 2 * L - 1  # 4095
N_CORES = 8
HPC = H // N_CORES  # heads per core
P = S + 1  # tile pitch (4096)

_cached_nc = None
DEFAULT_VARIANT = "pb_r4x16"


def _build_nc(variant=DEFAULT_VARIANT):
    import bass_rust
    import concourse.bacc as bacc
    import concourse.mybir as mybir
    import concourse.tile as tile

    host_exp = variant.startswith("hx")
    q3 = variant.endswith("q3")
    fused = "r4x4" in variant
    pipelined = variant.startswith(("hxp", "pbp"))

    nc = bacc.Bacc("TRN2", target_bir_lowering=False)
    f32 = mybir.dt.float32
    win = nc.dram_tensor("win", [HPC, P], f32, kind="ExternalInput")
    out = nc.dram_tensor("out", [HPC, L, L], f32, kind="ExternalOutput")

    if pipelined:
        # Per-(head, super-block) tiles so Tile's range-based dependency
        # tracking lets sb0's stores start while sb1..3 are still being
        # broadcast. Window of super-block sb: q columns
        # [1536-512sb, 4094-512sb]; store (sb, r) reads local columns
        # (511-r) - 4t + i, always inside [0, 2559).
        P2 = 2560
        with tile.TileContext(nc) as tc:
            with tc.tile_pool(name="p", bufs=1) as pool:
                k = 0

                def store(dst, sap):
                    nonlocal k
                    (nc.sync, nc.scalar)[k % 2].dma_start(dst, sap)
                    k += 1

                for h in range(HPC):
                    wt = pool.tile([1, P], f32, tag=f"wt{h}")
                    (nc.sync, nc.scalar)[h % 2].dma_start(
                        wt[:, :], win[h : h + 1, :]
                    )
                    if host_exp:
                        qrow = wt
                    else:
                        qrow = pool.tile([1, S], f32, tag=f"q1{h}")
                        nc.scalar.activation(
                            qrow[:, :],
                            wt[0:1, 0:S],
                            mybir.ActivationFunctionType.Exp,
                            bias=wt[0:1, S : S + 1],
                        )
                    for sb in range(4):
                        base = 1536 - 512 * sb
                        tsb = pool.tile([128, P2], f32, tag=f"t{h}_{sb}")
                        nc.gpsimd.partition_broadcast(
                            tsb[:, 0:2559], qrow[0:1, base : base + 2559]
                        )
                        for r in range(4):
                            sap = tsb[:, 0:L]
                            sap.ap = bass_rust.VecI64Pair([[P2 - 4, 128], [1, L]])
                            sap.offset = 511 - r
                            dst = out[h, 0:128, :]
                            dst.ap = bass_rust.VecI64Pair([[4 * L, 128], [1, L]])
                            dst.offset = (h * L + 512 * sb + r) * L
                            store(dst, sap)
        nc.compile()
        return nc

    with tile.TileContext(nc) as tc:
        with tc.tile_pool(name="p", bufs=1) as pool:
            qes = []
            for h in range(HPC):
                wt = pool.tile([1, P], f32, tag=f"wt{h}")
                qe = pool.tile([128, P], f32, tag=f"qe{h}")
                qes.append(qe)
                (nc.sync, nc.scalar)[h % 2].dma_start(
                    wt[:, :], win[h : h + 1, :]
                )
                if host_exp:
                    nc.gpsimd.partition_broadcast(qe[:, 0:S], wt[0:1, 0:S])
                else:
                    q1 = pool.tile([1, S], f32, tag=f"q1{h}")
                    nc.scalar.activation(
                        q1[:, :],
                        wt[0:1, 0:S],
                        mybir.ActivationFunctionType.Exp,
                        bias=wt[0:1, S : S + 1],
                    )
                    nc.gpsimd.partition_broadcast(qe[:, 0:S], q1[0:1, :])

            k = 0

            def store(dst, sap):
                nonlocal k
                engs = (
                    (nc.sync, nc.scalar, nc.gpsimd)
                    if q3
                    else (nc.sync, nc.scalar)
                )
                engs[k % len(engs)].dma_start(dst, sap)
                k += 1

            for h in range(HPC):
                if fused:
                    # 4 stores/head of 4 MB: sb fused, descending DRAM stride
                    for r in range(4):
                        c0 = L - 1 - 512 * 3 - r
                        sap = qes[h][:, 0:L]
                        sap.ap = bass_rust.VecI64Pair(
                            [[P - 4, 128], [512, 4], [1, L]]
                        )
                        sap.offset = c0
                        dst = out[h, 0:128, :]
                        dst.ap = bass_rust.VecI64Pair(
                            [[4 * L, 128], [-512 * L, 4], [1, L]]
                        )
                        dst.offset = (h * L + 512 * 3 + r) * L
                        store(dst, sap)
                else:
                    # 16 stores/head; partition t -> row 512*sb + 4t + r
                    for sb in range(4):
                        for r in range(4):
                            c0 = L - 1 - 512 * sb - r
                            sap = qes[h][:, 0:L]
                            sap.ap = bass_rust.VecI64Pair([[P - 4, 128], [1, L]])
                            sap.offset = c0
                            dst = out[h, 0:128, :]
                            dst.ap = bass_rust.VecI64Pair([[4 * L, 128], [1, L]])
                            dst.offset = (h * L + 512 * sb + r) * L
                            store(dst, sap)
    nc.compile()
    return nc


def _get_nc():
    global _cached_nc
    if _cached_nc is None:
        _cached_nc = _build_nc()
    return _cached_nc


def _make_in_maps(w_, offset, host_exp=False):
    w_ = np.asarray(w_, dtype=np.float32)
    offset = np.asarray(offset, dtype=np.float32)
    win = np.zeros((H, P), dtype=np.float32)
    if host_exp:
        win[:, 0:S] = np.exp(w_[:, ::-1] - offset[:, None])
    else:
        win[:, 0:S] = w_[:, ::-1]
        win[:, S] = -offset
    in_maps = []
    for c in range(N_CORES):
        sl = slice(c * HPC, (c + 1) * HPC)
        in_maps.append({"win": np.ascontiguousarray(win[sl])})
    return in_maps


def run(w_, offset, trace=False, variant=DEFAULT_VARIANT, **trace_kw):
    import concourse.bass_utils as bu
    from concourse.bass_utils import run_bass_kernel_spmd

    if trace:
        # no fish bucket in this container; keep artifacts local
        bu.upload_artifacts = lambda tmpdir: "local://" + str(tmpdir)

    if variant == DEFAULT_VARIANT:
        nc = _get_nc()
    else:
        nc = _build_nc(variant)
    in_maps = _make_in_maps(w_, offset, host_exp=variant.startswith("hx"))
    res = run_bass_kernel_spmd(
        nc, in_maps, list(range(N_CORES)), trace=trace, **trace_kw
    )
    parts = [np.asarray(r["out"]) for r in res.results]
    full = np.concatenate(parts, axis=0)  # [H, L, L]
    return full, res


def kernel(w_, offset, seq_len=None, **_ignored):
    full, _ = run(w_, offset, trace=False)
    return full


# revision 8
# speedup vs baseline: 1.4012x; 1.0048x over previous
"""Toeplitz bias kernel for trn2 (8 NeuronCores).

bias[h, j, i] = exp(w_[h] - offset[h])[2*L-2 + j - i]   with L = 2048.

Let q = reverse(exp(w_ - offset)) (length S = 2*L-1 = 4095); then
bias[h, j, i] = q[L-1 - j + i].

Device pipeline per head (no staircase, no chained small copies):
  1. load the packed 16 KB row [w_rev | -offset] into SBUF partition 0;
  2. exp on ACT over [1, S] (activation time is column-bound, so one
     partition costs the same as 128) with bias = -offset;
  3. gpsimd partition_broadcast replicates the exp'd row into a
     [128, S] tile -- engine-side, no DMA/HBM traffic;
  4. stores read that tile through a *diagonal* access pattern: giving
     dim0 a stride of (pitch - 4) makes partition t start 4 elements
     (16 B, line-aligned) earlier, so partition t supplies output row
     j = 512*sb + 4t + r and a [128, L] block store is one DMA:
        src[t, i] = qe[t, (L-1-512sb-r) - 4t + i]
     Four r-phases x four super-blocks = 16 store DMAs per head,
     alternated across the two HWDGE queues, ~16 MB in flight each.

The store phase is HBM-bound (~390 GB/s/core with all 8 cores writing);
everything else is off the critical path except ~10 us of load+exp+bcast.

Heads are sharded 2 per core across 8 cores; the host concatenates the
per-core [2, L, L] outputs.
"""

import numpy as np

H = 16
L = 2048
S = 2 * L - 1  # 4095
N_CORES = 8
HPC = H // N_CORES  # heads per core
P = S + 1  # tile pitch (4096)

_cached_nc = None
DEFAULT_VARIANT = "pb_r4x16"


def _build_nc(variant=DEFAULT_VARIANT):
    import bass_rust
    import concourse.bacc as bacc
    import concourse.mybir as mybir
    import concourse.tile as tile

    host_exp = variant.startswith("hx")
    q3 = variant.endswith("q3")
    fused = "r4x4" in variant
    pipelined = variant.startswith(("hxp", "pbp"))

    nc = bacc.Bacc("TRN2", target_bir_lowering=False)
    f32 = mybir.dt.float32
    win = nc.dram_tensor("win", [HPC, P], f32, kind="ExternalInput")
    out = nc.dram_tensor("out", [HPC, L, L], f32, kind="ExternalOutput")

    if pipelined:
        # Per-(head, super-block) tiles so Tile's range-based dependency
        # tracking lets sb0's stores start while sb1..3 are still being
        # broadcast. Window of super-block sb: q columns
        # [1536-512sb, 4094-512sb]; store (sb, r) reads local columns
        # (511-r) - 4t + i, always inside [0, 2559).
        P2 = 2560
        with tile.TileContext(nc) as tc:
            with tc.tile_pool(name="p", bufs=1) as pool:
                k = 0

                def store(dst, sap):
                    nonlocal k
                    (nc.sync, nc.scalar)[k % 2].dma_start(dst, sap)
                    k += 1

                for h in range(HPC):
                    wt = pool.tile([1, P], f32, tag=f"wt{h}")
                    (nc.sync, nc.scalar)[h % 2].dma_start(
                        wt[:, :], win[h : h + 1, :]
                    )
                    if host_exp:
                        qrow = wt
                    else:
                        qrow = pool.tile([1, S], f32, tag=f"q1{h}")
                        nc.scalar.activation(
                            qrow[:, :],
                            wt[0:1, 0:S],
                            mybir.ActivationFunctionType.Exp,
                            bias=wt[0:1, S : S + 1],
                        )
                    fuse2 = "f2" in variant
                    for sb in range(4):
                        base = 1536 - 512 * sb
                        tsb = pool.tile([128, P2], f32, tag=f"t{h}_{sb}")
                        nc.gpsimd.partition_broadcast(
                            tsb[:, 0:2559], qrow[0:1, base : base + 2559]
                        )
                        if fuse2:
                            # 2 stores/super-block: r-pairs fused via a
                            # negative middle stride on the SBUF side
                            for r0 in (0, 2):
                                sap = tsb[:, 0:L]
                                sap.ap = bass_rust.VecI64Pair(
                                    [[P2 - 4, 128], [-1, 2], [1, L]]
                                )
                                sap.offset = 511 - r0
                                dst = out[h, 0:128, :]
                                dst.ap = bass_rust.VecI64Pair(
                                    [[4 * L, 128], [L, 2], [1, L]]
                                )
                                dst.offset = (h * L + 512 * sb + r0) * L
                                store(dst, sap)
                        else:
                            for r in range(4):
                                sap = tsb[:, 0:L]
                                sap.ap = bass_rust.VecI64Pair(
                                    [[P2 - 4, 128], [1, L]]
                                )
                                sap.offset = 511 - r
                                dst = out[h, 0:128, :]
                                dst.ap = bass_rust.VecI64Pair(
                                    [[4 * L, 128], [1, L]]
                                )
                                dst.offset = (h * L + 512 * sb + r) * L
                                store(dst, sap)
        nc.compile()
        return nc

    with tile.TileContext(nc) as tc:
        with tc.tile_pool(name="p", bufs=1) as pool:
            qes = []
            for h in range(HPC):
                wt = pool.tile([1, P], f32, tag=f"wt{h}")
                qe = pool.tile([128, P], f32, tag=f"qe{h}")
                qes.append(qe)
                (nc.sync, nc.scalar)[h % 2].dma_start(
                    wt[:, :], win[h : h + 1, :]
                )
                if host_exp:
                    nc.gpsimd.partition_broadcast(qe[:, 0:S], wt[0:1, 0:S])
                else:
                    q1 = pool.tile([1, S], f32, tag=f"q1{h}")
                    nc.scalar.activation(
                        q1[:, :],
                        wt[0:1, 0:S],
                        mybir.ActivationFunctionType.Exp,
                        bias=wt[0:1, S : S + 1],
                    )
                    nc.gpsimd.partition_broadcast(qe[:, 0:S], q1[0:1, :])

            k = 0

            def store(dst, sap):
                nonlocal k
                engs = (
                    (nc.sync, nc.scalar, nc.gpsimd)
                    if q3
                    else (nc.sync, nc.scalar)
                )
                engs[k % len(engs)].dma_start(dst, sap)
                k += 1

            for h in range(HPC):
                if fused:
                    # 4 stores/head of 4 MB: sb fused, descending DRAM stride
                    for r in range(4):
                        c0 = L - 1 - 512 * 3 - r
                        sap = qes[h][:, 0:L]
                        sap.ap = bass_rust.VecI64Pair(
                            [[P - 4, 128], [512, 4], [1, L]]
                        )
                        sap.offset = c0
                        dst = out[h, 0:128, :]
                        dst.ap = bass_rust.VecI64Pair(
                            [[4 * L, 128], [-512 * L, 4], [1, L]]
                        )
                        dst.offset = (h * L + 512 * 3 + r) * L
                        store(dst, sap)
                else:
                    # 16 stores/head; partition t -> row 512*sb + 4t + r
                    for sb in range(4):
                        for r in range(4):
                            c0 = L - 1 - 512 * sb - r
                            sap = qes[h][:, 0:L]
                            sap.ap = bass_rust.VecI64Pair([[P - 4, 128], [1, L]])
                            sap.offset = c0
                            dst = out[h, 0:128, :]
                            dst.ap = bass_rust.VecI64Pair([[4 * L, 128], [1, L]])
                            dst.offset = (h * L + 512 * sb + r) * L
                            store(dst, sap)
    nc.compile()
    return nc


def _get_nc():
    global _cached_nc
    if _cached_nc is None:
        _cached_nc = _build_nc()
    return _cached_nc


def _make_in_maps(w_, offset, host_exp=False):
    w_ = np.asarray(w_, dtype=np.float32)
    offset = np.asarray(offset, dtype=np.float32)
    win = np.zeros((H, P), dtype=np.float32)
    if host_exp:
        win[:, 0:S] = np.exp(w_[:, ::-1] - offset[:, None])
    else:
        win[:, 0:S] = w_[:, ::-1]
        win[:, S] = -offset
    in_maps = []
    for c in range(N_CORES):
        sl = slice(c * HPC, (c + 1) * HPC)
        in_maps.append({"win": np.ascontiguousarray(win[sl])})
    return in_maps


def run(w_, offset, trace=False, variant=DEFAULT_VARIANT, **trace_kw):
    import concourse.bass_utils as bu
    from concourse.bass_utils import run_bass_kernel_spmd

    if trace:
        # no fish bucket in this container; keep artifacts local
        bu.upload_artifacts = lambda tmpdir: "local://" + str(tmpdir)

    if variant == DEFAULT_VARIANT:
        nc = _get_nc()
    else:
        nc = _build_nc(variant)
    in_maps = _make_in_maps(w_, offset, host_exp=variant.startswith("hx"))
    res = run_bass_kernel_spmd(
        nc, in_maps, list(range(N_CORES)), trace=trace, **trace_kw
    )
    parts = [np.asarray(r["out"]) for r in res.results]
    full = np.concatenate(parts, axis=0)  # [H, L, L]
    return full, res


def kernel(w_, offset, seq_len=None, **_ignored):
    full, _ = run(w_, offset, trace=False)
    return full
